# revision 31
# baseline (speedup 1.0000x reference)
"""Trainium2 Bass kernel for masked attention (nn_Attention_77704548319841).

Reference computation per batch b:
    CI     = einsum('sc,hc->hs', context[b], W_a)          # [H, S]
    scores = einsum('th,hs->ts', dec[b], CI)               # [T, S]
    scores = where(mask, -1e6, scores)
    attn   = softmax(scores, axis=-1)
    out[b] = einsum('ts,sc->tc', attn, context[b])         # [T, 2H]

Sharding: pure data parallel over batch (16 batches / 8 cores = 2 per core).

v5 design: natural-layout softmax + half-fp8 mm3 (DoubleRow).
  - Host supplies SBUF-image operands:
      waT  [C, H] f16, ctxT [B, C, S] f16 (mm1), decT [B, H, T] f16 (mm2)
      maskN[B, T, S] u8   (natural layout - softmax is per t-row now)
      ctx8 [B, s<512, C] fp8e4 (mm3 rhs, DoubleRow pair-major)
      ctxN [B, s>=512, C] f16  (mm3 rhs, bf16-path half)
      id8/idb: 128x128 identities for PE transposes
  - mm1: CI[h, s] = waT.T @ ctxT  (as before)
  - mm2: NATURAL scores[t, s] = decT.T @ CI per t-tile; softmax along the
    FREE dim: mask-fuse (DVE stt), row max (DVE reduce), exp with
    per-partition bias=-max (ACT) emitting attn in fp8e4 for s<512 and bf16
    for s>=512; rowsum via DVE free-dim reduces (NO rowsum matmuls at all);
    reciprocal ready before mm3 even starts.
  - attn tiles are PE-transposed ([t,s]->[s,t], 8x 128x128 per t-tile) into
    DoubleRow-pair-major SBUF tiles.  Transposes cost 1 cycle/row; their
    psum tiles are 1 bank each and evict while the next tile's scores run.
  - mm3: per t-tile, ONE f32 psum accumulates the fp8 half (2 DoubleRow
    matmuls, K=256 each, 2x throughput) then the bf16 half (4 matmuls);
    eviction applies 1/rowsum.  The s<512 half runs at 157 TF/s vs 78.6.
  - Precision: attn-fp8 error largely cancels through the rowsum (computed
    from the quantized values); ctx-fp8 on half the contraction gives
    ~1.9e-2 Frobenius rel err vs the 2e-2 budget (measured in emulation;
    deterministic on HW).
  - Startup/tail scheduling as v4: bulk loads on one gpsimd queue in strict
    consumption-priority order (+7us if split across queues - the DMA
    aggregate is hw-fixed), waT parallel on sync, constants on vector,
    16 HAM warmups bridging the DMA ramp, last tile accumulates c-low
    before c-high so only one 256KB store trails the final matmul.
"""

import numpy as np
import ml_dtypes
from contextlib import ExitStack

import concourse.bass as bass
import concourse.tile as tile
from concourse import bacc, mybir
from concourse.bass_utils import run_bass_kernel_spmd

B, T, S, H = 16, 1024, 1024, 512
C = 2 * H
N_CORES = 8
BLOC = B // N_CORES  # batches per core
P = 128
NT = T // P   # 8 t-tiles
NS = S // P   # 8 s-tiles
NH = H // P   # 4 h-tiles
NC_ = C // P  # 8 c-tiles
FD = 512      # matmul free-dim chunk
F8S = 512     # s-range handled in fp8 (must be multiple of 256)
NJ8 = F8S // P        # 4 fp8 s-tiles
NJB = (S - F8S) // P  # 4 bf16 s-tiles
NEG_BIG = -1.0e6
N_WARMUP = 16

f32 = mybir.dt.float32
f16 = mybir.dt.float16
bf16 = mybir.dt.bfloat16
f8e4 = mybir.dt.float8e4
u8 = mybir.dt.uint8
AF = mybir.ActivationFunctionType
ALU = mybir.AluOpType
AX = mybir.AxisListType
DR = mybir.MatmulPerfMode.DoubleRow


def _emit(ctx: ExitStack, tc: "tile.TileContext", out_d, decT_d, ctxT_d, ctxN_d,
          ctx8_d, maskN_d, waT_d, id8_d, idb_d):
    nc = tc.nc

    pw = ctx.enter_context(tc.tile_pool(name="pw", bufs=1))
    pin = ctx.enter_context(tc.tile_pool(name="pin", bufs=1))
    ptmp = ctx.enter_context(tc.tile_pool(name="ptmp", bufs=2))
    pout = ctx.enter_context(tc.tile_pool(name="pout", bufs=2))
    pstat = ctx.enter_context(tc.tile_pool(name="pstat", bufs=2))
    ppsum = ctx.enter_context(
        tc.tile_pool(name="ppsum", bufs=2, space=bass.MemorySpace.PSUM)
    )

    wz = pw.tile([P, FD], f16, tag="wz")

    # ---- persistent input tiles --------------------------------------------
    waT = pw.tile([P, NH, NC_, P], f16, tag="waT")      # waT[p, mh, ct, h]
    id8 = pw.tile([P, P], f8e4, tag="id8")
    idb = pw.tile([P, P], bf16, tag="idb")
    ctxT = [pin.tile([P, 4, NC_, 256], f16, tag=f"ctxT{b}", name=f"ctxT{b}")
            for b in range(BLOC)]
    ctxN = [pin.tile([P, NJB, C], f16, tag=f"ctxN{b}", name=f"ctxN{b}")
            for b in range(BLOC)]
    ctx8 = [pin.tile([P, NJ8, C], f8e4, tag=f"ctx8{b}", name=f"ctx8{b}")
            for b in range(BLOC)]
    decT = [pin.tile([P, NH, T], f16, tag=f"decT{b}", name=f"decT{b}")
            for b in range(BLOC)]
    maskN = [pin.tile([P, NT, S], u8, tag=f"maskN{b}", name=f"maskN{b}")
             for b in range(BLOC)]
    # transposed attention, DoubleRow-pair-major on dim1
    attnT8 = [pin.tile([P, NJ8, T], f8e4, tag=f"aT8{b}", name=f"aT8{b}")
              for b in range(BLOC)]
    attnTb = [pin.tile([P, NJB, T], bf16, tag=f"aTb{b}", name=f"aTb{b}")
              for b in range(BLOC)]

    wa_r = waT_d.rearrange("p (mh ct h) -> p mh ct h", mh=NH, ct=NC_)
    cr = [ctxT_d[b].rearrange("p (q ct s) -> p q ct s", q=4, ct=NC_)
          for b in range(BLOC)]
    dr = [decT_d[b].rearrange("p (kh t) -> p kh t", kh=NH) for b in range(BLOC)]
    nr = [ctxN_d[b].rearrange("p (st c) -> p st c", st=NJB) for b in range(BLOC)]
    er = [ctx8_d[b].rearrange("p (st c) -> p st c", st=NJ8) for b in range(BLOC)]
    mr = [maskN_d[b].rearrange("p (tt s) -> p tt s", tt=NT) for b in range(BLOC)]

    # vector: constants only (keeps gpsimd/scalar queues free at startup)
    nc.vector.memset(wz[:], 0.0)

    # gpsimd: every bulk load, strict consumption-priority order (one bulk
    # queue is optimal: the DMA aggregate is hw-fixed and extra queues
    # round-robin bandwidth away from the critical ctxT0 stream)
    nc.gpsimd.dma_start(ctxT[0][:, 0, 0:4], cr[0][:, 0, 0:4])
    nc.gpsimd.dma_start(ctxT[0][:, 0, 4:8], cr[0][:, 0, 4:8])
    nc.gpsimd.dma_start(ctxT[0][:, 1], cr[0][:, 1])
    nc.gpsimd.dma_start(ctxT[0][:, 2], cr[0][:, 2])
    nc.gpsimd.dma_start(ctxT[0][:, 3], cr[0][:, 3])
    nc.gpsimd.dma_start(decT[0][:], dr[0][:])
    nc.gpsimd.dma_start(maskN[0][:], mr[0][:])
    nc.gpsimd.dma_start(ctxT[1][:], cr[1][:])
    nc.gpsimd.dma_start(ctx8[0][:], er[0][:])
    nc.gpsimd.dma_start(ctxN[0][:], nr[0][:])
    nc.gpsimd.dma_start(decT[1][:], dr[1][:])
    nc.gpsimd.dma_start(maskN[1][:], mr[1][:])
    nc.gpsimd.dma_start(ctx8[1][:], er[1][:])
    nc.gpsimd.dma_start(ctxN[1][:], nr[1][:])

    # sync: waT (parallel, mh-granular) + identities + stores later
    nc.sync.dma_start(waT[:, 0], wa_r[:, 0])
    nc.sync.dma_start(waT[:, 1], wa_r[:, 1])
    nc.sync.dma_start(waT[:, 2], wa_r[:, 2])
    nc.sync.dma_start(waT[:, 3], wa_r[:, 3])
    nc.sync.dma_start(id8[:], id8_d)
    nc.sync.dma_start(idb[:], idb_d)

    # ---- PE warm-up (HAM) while loads land ---------------------------------
    wps = ppsum.tile([P, FD], f32, tag="psh", bufs=2, name="warm0")
    for _ in range(N_WARMUP):
        nc.tensor.matmul(wps[:], wz[:, 0:P], wz[:], start=True, stop=True)

    # ---- per-batch state ----------------------------------------------------
    CI = [None] * BLOC     # [p, kh, s] fp16 (natural: partitions = h)
    rr = [None] * BLOC     # [p(t), tc] f32 reciprocal rowsums

    def mm1(b):
        """CI[h, s] = W_a @ ctx[b].T  (accumulate over c)."""
        ci = ptmp.tile([P, NH, S], f16, tag=f"CI{b}", bufs=1, name=f"CI{b}")
        CI[b] = ci
        if b == 0:
            # ns0 in s-quarters: matches the DMA engine ramp so the PE starts
            # earlier and stays dense
            for q in range(2):
                qsl = slice(q * 256, q * 256 + 256)
                for mh in range(NH):
                    psq = ppsum.tile([P, FD], f32, tag="psh", bufs=2,
                                     name="psq")
                    for ct in range(NC_):
                        nc.tensor.matmul(
                            psq[:, 0:256],
                            waT[:, mh, ct, :],
                            ctxT[b][:, q, ct, :],
                            start=(ct == 0),
                            stop=(ct == NC_ - 1),
                        )
                    nc.vector.tensor_copy(ci[:, mh, qsl], psq[:, 0:256])
        ns_range = [1] if b == 0 else [0, 1]
        for ns in ns_range:
            for mh in range(NH):
                ps = ppsum.tile([P, FD], f32, tag="psh", bufs=2, name="psh")
                for ct in range(NC_):
                    nc.tensor.matmul(
                        ps[:],
                        waT[:, mh, ct, :],
                        ctxT[b][:, 2 * ns : 2 * ns + 2, ct, :],
                        start=(ct == 0),
                        stop=(ct == NC_ - 1),
                    )
                nc.vector.tensor_copy(ci[:, mh, ns * FD : ns * FD + FD], ps[:])

    def mm2_softmax(b):
        """Natural scores[t, s] per t-tile; per-row softmax; transposed
        fp8/bf16 attention tiles + reciprocal rowsums."""
        rrt = pstat.tile([P, NT], f32, tag=f"rr{b}", bufs=1, name=f"rr{b}")
        rr[b] = rrt
        for mt in range(NT):
            tsl = slice(mt * P, (mt + 1) * P)
            ps = ppsum.tile([P, S], f32, tag="ps", bufs=2, name="ps")
            for kh in range(NH):
                lhs = decT[b][:, kh, tsl]
                for th in range(2):
                    nc.tensor.matmul(
                        ps[:, th * FD : (th + 1) * FD],
                        lhs,
                        CI[b][:, kh, th * FD : (th + 1) * FD],
                        start=(kh == 0),
                        stop=(kh == NH - 1),
                    )
            sm = ptmp.tile([P, S], f32, tag="sm", bufs=2, name="sm")
            nc.vector.scalar_tensor_tensor(
                sm[:], maskN[b][:, mt, :], NEG_BIG, ps[:], op0=ALU.mult,
                op1=ALU.add,
            )
            nmx = pstat.tile([P, 2], f32, tag="nmx", bufs=2, name="nmx")
            nc.vector.tensor_reduce(nmx[:, 0:1], sm[:], AX.X, ALU.max)
            nc.vector.tensor_scalar_mul(nmx[:, 1:2], nmx[:, 0:1], -1.0)
            a8 = ptmp.tile([P, F8S], f8e4, tag="a8", bufs=2, name="a8")
            nc.scalar.activation(a8[:], sm[:, 0:F8S], AF.Exp,
                                 bias=nmx[:, 1:2], scale=1.0)
            ab = ptmp.tile([P, S - F8S], bf16, tag="ab", bufs=2, name="ab")
            nc.scalar.activation(ab[:], sm[:, F8S:S], AF.Exp,
                                 bias=nmx[:, 1:2], scale=1.0)
            # rowsum from the QUANTIZED attn (error cancellation) via
            # free-dim reduces - no PE rowsum matmuls at all
            rsum = pstat.tile([P, 2], f32, tag="rsum", bufs=2, name="rsum")
            nc.vector.tensor_reduce(rsum[:, 0:1], a8[:], AX.X, ALU.add)
            nc.vector.tensor_reduce(rsum[:, 1:2], ab[:], AX.X, ALU.add)
            nc.vector.tensor_tensor(rsum[:, 0:1], rsum[:, 0:1], rsum[:, 1:2],
                                    ALU.add)
            nc.vector.reciprocal(rrt[:, mt : mt + 1], rsum[:, 0:1])
            # PE transposes [t,s]->[s,t]; all 4 slices of one dtype land in
            # one 1-bank psum tile, evicted in a single strided copy
            # fp8 transpose writes on 2-byte granularity: out AP elem step 2
            pt8 = ppsum.tile([P, NJ8, P, 2], f8e4, tag="pt8", bufs=1,
                             name="pt8")
            for j in range(NJ8):
                nc.tensor.transpose(pt8[:, j, :, 0],
                                    a8[:, j * P : (j + 1) * P], id8[:])
            nc.vector.tensor_copy(attnT8[b][:, :, tsl], pt8[:, :, :, 0])
            ptb = ppsum.tile([P, NJB, P], bf16, tag="ptb", bufs=1, name="ptb")
            for j in range(NJB):
                nc.tensor.transpose(ptb[:, j], ab[:, j * P : (j + 1) * P],
                                    idb[:])
            nc.scalar.activation(attnTb[b][:, :, tsl], ptb[:], AF.Copy,
                                 bias=0.0, scale=1.0)

    def mm3(b):
        """out[t, c] = attn @ ctx: fp8 DoubleRow for s<512, bf16 for the
        rest, one f32 psum accumulation; eviction applies 1/rowsum."""
        rrt = rr[b]
        orr = out_d[b].rearrange("(tt p) c -> p tt c", p=P)
        for mt in range(NT):
            last = b == BLOC - 1 and mt == NT - 1
            ps = ppsum.tile([P, C], f32, tag="ps", bufs=2, name="ps")
            tsl = slice(mt * P, (mt + 1) * P)

            def half(nck, dst):
                for j in range(NJ8 // 2):
                    nc.tensor.matmul(
                        dst,
                        attnT8[b][:, 2 * j : 2 * j + 2, tsl],
                        ctx8[b][:, 2 * j : 2 * j + 2,
                                nck * FD : (nck + 1) * FD],
                        start=(j == 0), stop=False, perf_mode=DR,
                    )
                for k in range(NJB):
                    nc.tensor.matmul(
                        dst,
                        attnTb[b][:, k, tsl],
                        ctxN[b][:, k, nck * FD : (nck + 1) * FD],
                        start=False, stop=(k == NJB - 1),
                    )

            ph = ppsum.tile([P, FD], f32, tag="psh", bufs=2,
                            name="ph") if last else None
            if last:
                # c-low fully first: its store flushes before the end
                half(0, ps[:, 0:FD])
                half(1, ph[:])
            else:
                half(0, ps[:, 0:FD])
                half(1, ps[:, FD:C])
            ob = pout.tile([P, C], f16, tag="ob", bufs=2, name="ob")
            if last:
                ob2 = pout.tile([P, FD], f16, tag="ob2", bufs=1, name="ob2")
                nc.scalar.activation(ob[:, 0:FD], ps[:, 0:FD], AF.Copy,
                                     bias=0.0, scale=rrt[:, mt : mt + 1])
                nc.sync.dma_start(orr[:, mt, 0:FD], ob[:, 0:FD])
                nc.scalar.activation(ob2[:, 0:256], ph[:, 0:256], AF.Copy,
                                     bias=0.0, scale=rrt[:, mt : mt + 1])
                nc.vector.tensor_scalar_mul(ob2[:, 256:FD], ph[:, 256:FD],
                                            rrt[:, mt : mt + 1])
                nc.gpsimd.dma_start(orr[:, mt, FD : FD + 256], ob2[:, 0:256])
                nc.sync.dma_start(orr[:, mt, FD + 256 : C], ob2[:, 256:FD])
            elif mt % 2 == 0:
                nc.scalar.activation(ob[:], ps[:], AF.Copy, bias=0.0,
                                     scale=rrt[:, mt : mt + 1])
                nc.sync.dma_start(orr[:, mt, :], ob[:])
            else:
                nc.vector.tensor_scalar_mul(ob[:], ps[:], rrt[:, mt : mt + 1])
                nc.sync.dma_start(orr[:, mt, :], ob[:])

    mm1(0)
    mm2_softmax(0)
    mm1(1)
    mm3(0)
    mm2_softmax(1)
    mm3(1)


_BUILT = None


def _build():
    global _BUILT
    if _BUILT is not None:
        return _BUILT
    nc = bacc.Bacc("TRN2", target_bir_lowering=False, debug=False)
    decT_d = nc.dram_tensor("decT", [BLOC, P, NH * T], f16, kind="ExternalInput")
    ctxT_d = nc.dram_tensor("ctxT", [BLOC, P, C * S // P], f16, kind="ExternalInput")
    ctxN_d = nc.dram_tensor("ctxN", [BLOC, P, NJB * C], f16, kind="ExternalInput")
    ctx8_d = nc.dram_tensor("ctx8", [BLOC, P, NJ8 * C], f8e4, kind="ExternalInput")
    maskN_d = nc.dram_tensor("maskN", [BLOC, P, NT * S], u8, kind="ExternalInput")
    waT_d = nc.dram_tensor("waT", [P, C * H // P], f16, kind="ExternalInput")
    id8_d = nc.dram_tensor("id8", [P, P], f8e4, kind="ExternalInput")
    idb_d = nc.dram_tensor("idb", [P, P], bf16, kind="ExternalInput")
    out_d = nc.dram_tensor("out", [BLOC, T, C], f16, kind="ExternalOutput")
    with tile.TileContext(nc) as tc, ExitStack() as ctx:
        _emit(ctx, tc, out_d.ap(), decT_d.ap(), ctxT_d.ap(), ctxN_d.ap(),
              ctx8_d.ap(), maskN_d.ap(), waT_d.ap(), id8_d.ap(), idb_d.ap())
    nc.compile()
    _BUILT = nc
    return nc


def make_in_maps(decoder_output, context, mask, W_a):
    dec = np.asarray(decoder_output, dtype=np.float32)
    ctx = np.asarray(context, dtype=np.float32)
    msk = np.asarray(mask)
    wa = np.asarray(W_a, dtype=np.float32)
    e4 = ml_dtypes.float8_e4m3fn

    # decT tile [p, kh, t] = dec[b, t, kh*128+p]
    decT = np.ascontiguousarray(
        dec.transpose(0, 2, 1).reshape(B, NH, P, T).transpose(0, 2, 1, 3)
        .reshape(B, P, NH * T).astype(np.float16))
    # ctxT tile [p, q, ct, s4] = ctx[b, q*256+s4, ct*128+p]  (quarter-major)
    ctxT = np.ascontiguousarray(
        ctx.transpose(0, 2, 1).reshape(B, NC_, P, 4, 256)
        .transpose(0, 2, 3, 1, 4).reshape(B, P, C * S // P).astype(np.float16))
    # ctxN tile [p, st, c] = ctx[b, 512 + st*128+p, c]  (bf16-path half)
    ctxN = np.ascontiguousarray(
        ctx[:, F8S:].reshape(B, NJB, P, C).transpose(0, 2, 1, 3)
        .reshape(B, P, NJB * C).astype(np.float16))
    # ctx8 tile [p, st, c] = e4m3(ctx[b, st*128+p, c]) for s < 512
    ctx8 = np.ascontiguousarray(
        ctx[:, :F8S].reshape(B, NJ8, P, C).transpose(0, 2, 1, 3)
        .reshape(B, P, NJ8 * C).astype(e4))
    # maskN tile [p, tt, s] = mask[b, tt*128+p, s]  (natural layout)
    maskN = np.ascontiguousarray(
        msk.reshape(B, NT, P, S).transpose(0, 2, 1, 3)
        .reshape(B, P, NT * S)).astype(np.uint8)
    # waT tile [p, mh, ct, h2] = W_a[mh*128+h2, ct*128+p]
    waT = np.ascontiguousarray(
        wa.T.reshape(NC_, P, NH, P).transpose(1, 2, 0, 3)
        .reshape(P, C * H // P).astype(np.float16))
    id8 = np.eye(P, dtype=e4)
    idb = np.eye(P, dtype=ml_dtypes.bfloat16)

    in_maps = []
    for i in range(N_CORES):
        sl = slice(i * BLOC, (i + 1) * BLOC)
        in_maps.append(
            {
                "decT": decT[sl],
                "ctxT": ctxT[sl],
                "ctxN": ctxN[sl],
                "ctx8": ctx8[sl],
                "maskN": maskN[sl],
                "waT": waT,
                "id8": id8,
                "idb": idb,
            }
        )
    return in_maps


def kernel(decoder_output, context, mask, W_a, **run_kwargs):
    nc = _build()
    in_maps = make_in_maps(decoder_output, context, mask, W_a)
    res = run_bass_kernel_spmd(nc, in_maps, core_ids=list(range(N_CORES)), **run_kwargs)
    out = np.concatenate([res.results[i]["out"] for i in range(N_CORES)], axis=0)
    return out.astype(np.float32)


if __name__ == "__main__":
    nc = _build()
    print("build + compile OK")
